# revision 26
# baseline (speedup 1.0000x reference)
"""Trainium2 Bass kernel for the DeformableSDFHead MLP.

Network (per point n, 16 bones k):
  x_k = [xyz3_k (3) | gl (48)]            gl shared per group of 4096 points
  h0  = relu(W0_k x_k + b0_k)             51 -> 64
  h_{l+1} = relu(Wmid_{k,l} h_l + bmid + h_l)   l = 0..6  (residual)
  latent = pre-residual out of l=6
  y = tanh(concat_k(latent_k) . Wf + bf)

Restructuring (all folds done host-side in numpy):
  * gl part of layer 0 folded into a per-(bone, group) bias beff.
  * residual folded into weights: W' = Wmid + I for l=0..5.
  * layer l=6 + final projection folded into a per-bone matvec:
      y = tanh(sum_k v_k . h6_k + c),  v_k = Wmid_{k,6}^T Wf_k.

Mapping: data-parallel over 8 cores (8192 points each). Per core, bones are
packed 2-at-a-time onto 128 partitions; mid-layer GEMMs run as 4 concurrent
64x64 quadrant tiles, layer 0 as 8-row-contraction (32,64) tiles over the
same quadrant positions. Activations fp16, psum fp32.

The kernel is PSUM-evacuation-bound: only ScalarE (1.2GHz) and VectorE
(0.96GHz) can read PSUM, at 1 elem/cycle/partition for fp32, so the
7 x 16 x 64 x 8192 relu'd activations per core set a ~255us floor on the two
engines' busy time. Everything else (PE matmuls ~210us, DMA ~30us) hides
under that. Relu+bias evacuation is split 239:209 across ScalarE (activation
Relu w/ bias) and VectorE (tensor_scalar add+max, which also carries the 4
matvec copies), matching their measured per-op rates (~1.07us vs ~1.22us per
[128,1024] tile). The per-sg matvec is scheduled mid-way through the next
sg's first mid layer -- chains split in two, PSUM-accumulator copy deferred
-- so its psum ring slot never gates the drain engines.
"""

import numpy as np

import concourse.bacc as bacc
import concourse.bass as bass
import concourse.mybir as mybir
from concourse.tile import TileContext
from concourse.bass_utils import run_bass_kernel_spmd

NUM_BONES = 16
HID = 64
JOINT_IDX = np.array([0, 1, 2, 3, 5, 6, 7, 9, 10, 11, 13, 14, 15, 17, 18, 19])

NCORES = 8
N = 65536
NS = N // NCORES       # 8192 points per core
SG = 2048              # supergroup (points held in SBUF per pipeline stage)
NSG = NS // SG         # 4
F = 512                # matmul free-dim chunk (one psum bank)
RF = 1024              # relu op free dim (2 psum banks)
KX = 8                 # layer-0 contraction: rows 0:3 bone 4g, 4:7 bone 4g+3
                       # (w0's off-slot rows are zero, so the 8-row contraction
                       # is exact; no zero-padding/memset needed -- the PE
                       # tile-mode switch to (32,64) is harmless since the
                       # matvec already switches modes every sg)

FP16 = mybir.dt.float16
FP32 = mybir.dt.float32

_SIGMA = [[(2 * p, 2 * p + 1) for p in range(8)]]
for _l in range(6):
    _SIGMA.append([_SIGMA[-1][p] if p % 2 == 0 else _SIGMA[-1][p][::-1]
                   for p in range(8)])


def _host_prep(xyz, joints, W0, b0, Wmid, bmid, Wf, bf):
    f32 = np.float32
    B = joints.shape[0]
    gl = joints[:, JOINT_IDX, :]
    gl = (gl - gl[:, :1, :]).reshape(B, -1).astype(f32)

    W0a = W0[:, :, 0:3].astype(f32)
    W0b = W0[:, :, 3:].astype(f32)
    beff = b0[:, None, :].astype(f32) + np.einsum('gi,koi->kgo', gl, W0b)

    I = np.eye(HID, dtype=f32)
    Wm_f = Wmid[:, :6].astype(f32) + I[None, None]

    Wf_k = Wf.reshape(NUM_BONES, HID).astype(f32)
    v = np.einsum('koi,ko->ki', Wmid[:, 6].astype(f32), Wf_k)
    c = float(np.sum(bmid[:, 6].astype(f32) * Wf_k) + bf[0])

    wm = np.zeros((128, 6 * 8 * 64), dtype=f32)
    bm = np.zeros((128, 48), dtype=f32)
    for l in range(6):
        for p in range(8):
            col = (l * 8 + p) * 64
            blo, bhi = _SIGMA[l][p]
            wm[0:64, col:col + 64] = Wm_f[blo, l].T
            wm[64:128, col:col + 64] = Wm_f[bhi, l].T
            olo, ohi = _SIGMA[l + 1][p]
            bm[0:64, l * 8 + p] = bmid[olo, l]
            bm[64:128, l * 8 + p] = bmid[ohi, l]

    # layer-0 weights in the same 4-quadrant layout as the mids. For group g
    # (bones 4g..4g+3), pair 2g is "straight" (diag quadrants) and pair 2g+1
    # "swapped" (anti-diag), matching _SIGMA[0]:
    #   Q(0,0)  : rhs rows 0:3   (bone 4g)   -> psA parts 0:64
    #   Q(64,64): rhs rows 64:67 (bone 4g+1) -> psA parts 64:128
    #   Q(64,0) : rhs rows 68:71 (bone 4g+2) -> psB parts 0:64
    #   Q(0,64) : rhs rows 4:7   (bone 4g+3) -> psB parts 64:128
    w0 = np.zeros((128, 512), dtype=f32)
    for g in range(4):
        w0[0:3, 128 * g:128 * g + 64] = W0a[4 * g].T
        w0[64 + 0:64 + 3, 128 * g + 64:128 * g + 128] = W0a[4 * g + 1].T
        w0[64 + 4:64 + 7, 128 * g:128 * g + 64] = W0a[4 * g + 2].T
        w0[4:7, 128 * g + 64:128 * g + 128] = W0a[4 * g + 3].T

    vt = np.zeros((128, 8 * 32), dtype=f32)
    for p in range(8):
        blo, bhi = _SIGMA[6][p]
        vt[0:64, 32 * p:32 * p + 32] = v[blo][:, None]
        vt[64:128, 32 * p:32 * p + 32] = v[bhi][:, None]

    xyzf = xyz.astype(f32)
    in_maps = []
    for core in range(NCORES):
        n0 = core * NS
        x3 = np.zeros((4, 16, NS), dtype=np.float16)
        for g in range(4):
            # rows 0:3 -> xt[0:3], 4:7 -> xt[4:7], 8:11 -> xt[64:67],
            # 12:15 -> xt[68:71]
            for slot, b_ in ((0, 4 * g), (1, 4 * g + 3),
                             (2, 4 * g + 1), (3, 4 * g + 2)):
                x3[g, 4 * slot:4 * slot + 3, :] = (
                    xyzf[n0:n0 + NS, 3 * (b_ + 1):3 * (b_ + 1) + 3].T
                    .astype(np.float16))
        b0e = np.zeros((128, 16), dtype=f32)
        for p in range(8):
            blo, bhi = _SIGMA[0][p]
            for gi in range(2):
                grp = 2 * core + gi
                b0e[0:64, p * 2 + gi] = beff[blo, grp]
                b0e[64:128, p * 2 + gi] = beff[bhi, grp]
        in_maps.append(dict(
            x3=x3,
            w0=w0.astype(np.float16),
            wm=wm.astype(np.float16),
            bm=bm,
            b0e=b0e,
            vt=vt.astype(np.float16),
        ))
    return in_maps, c


_CACHE = {}


def _build():
    nc = bacc.Bacc("TRN2", target_bir_lowering=False)

    x3_h = nc.dram_tensor("x3", [4, 16, NS], FP16, kind="ExternalInput")
    w0_h = nc.dram_tensor("w0", [128, 512], FP16, kind="ExternalInput")
    wm_h = nc.dram_tensor("wm", [128, 6 * 8 * 64], FP16, kind="ExternalInput")
    bm_h = nc.dram_tensor("bm", [128, 48], FP32, kind="ExternalInput")
    b0e_h = nc.dram_tensor("b0e", [128, 16], FP32, kind="ExternalInput")
    vt_h = nc.dram_tensor("vt", [128, 8 * 32], FP16, kind="ExternalInput")
    out_h = nc.dram_tensor("out", [NSG, 4, F], FP32, kind="ExternalOutput")

    Relu = mybir.ActivationFunctionType.Relu
    ADD = mybir.AluOpType.add
    MAX = mybir.AluOpType.max

    # relu engine split by measured rates (ACT ~1071ns vs DVE ~1215ns per op,
    # DVE also carrying the 4 matvec copies): ACT gets exactly 239 of the 448
    # relus per core, spread evenly.
    act_pick = [((i * 239) % 448) < 239 for i in range(448)]

    with TileContext(nc) as tc:
        with (
            tc.tile_pool(name="const", bufs=1) as cpool,
            tc.tile_pool(name="hbuf", bufs=2) as hpool,
            tc.tile_pool(name="outp", bufs=2) as opool,
            tc.tile_pool(name="ps", bufs=4, space="PSUM") as pspool,
        ):
            w0_t = cpool.tile([128, 512], FP16, name="w0t")
            wm_t = cpool.tile([128, 6 * 8 * 64], FP16, name="wmt")
            bm_t = cpool.tile([128, 48], FP32, name="bmt")
            b0e_t = cpool.tile([128, 16], FP32, name="b0et")
            vt_t = cpool.tile([128, 8 * 32], FP16, name="vtt")
            # persistent double-buffered x tiles: DMAs write rows 0:8 and
            # 64:72; the KX=8 contraction reads only those rows (the rest of
            # each tile is never touched).
            xg_t = [[cpool.tile([128, SG], FP16, name=f"x{g}b{b}")
                     for b in range(2)] for g in range(4)]

            nc.sync.dma_start(out=w0_t[:, :], in_=w0_h[:, :])
            nc.sync.dma_start(out=b0e_t[:, :], in_=b0e_h[:, :])
            for g in range(4):
                nc.sync.dma_start(out=xg_t[g][0][0:8, :],
                                  in_=x3_h[g, 0:8, 0:SG])
                nc.sync.dma_start(out=xg_t[g][0][64:72, :],
                                  in_=x3_h[g, 8:16, 0:SG])
            for l in range(6):
                nc.sync.dma_start(out=wm_t[:, 512 * l:512 * (l + 1)],
                                  in_=wm_h[:, 512 * l:512 * (l + 1)])
            nc.sync.dma_start(out=bm_t[:, :], in_=bm_h[:, :])
            nc.sync.dma_start(out=vt_t[:, :], in_=vt_h[:, :])

            relu_i = 0
            mv_inflight = None
            pending_mv = None  # delay matvec past next sg's layer 0 so the
                               # PE stays fed while the prior sg's last relus
                               # drain; the L0 matmuls (which depend on
                               # nothing) must all sit BEFORE the matvec in
                               # the in-order PE queue or they get blocked
                               # behind its relu-wait and starve ACT/DVE

            def emit_mv_chains(h6, mv, p_lo, p_hi):
                # 4 col-concurrent accumulation chains (one per cc), p-outer
                # so the chains interleave and overlap in the PE array.
                for p in range(p_lo, p_hi):
                    for cc in range(4):
                        nc.tensor.matmul(
                            out=mv[32 * cc:32 * cc + 32, :],
                            lhsT=vt_t[:, 32 * p:32 * p + 32],
                            rhs=h6[p][:, cc * F:(cc + 1) * F],
                            start=(p == 0), stop=(p == 7),
                            tile_position=(0, 32 * cc),
                            skip_group_check=True)

            def emit_mv_out(mv, msg):
                # on DVE: cheaper there than ACT's activation-copy (~617 vs
                # ~686ns) and ACT is the busier engine.
                out_sb = opool.tile([128, F], FP32, name="osb", tag="osb")
                nc.vector.tensor_copy(out_sb[0:97, :], mv[0:97, :])
                ou_v = out_sb.rearrange("(a b) f -> a b f", b=32)[:, 0:1, :]
                nc.sync.dma_start(out=out_h[msg, :, :], in_=ou_v)

            def emit_matvec(h6, msg, mv=None, copy_now=True):
                if mv is None:
                    mv = pspool.tile([128, RF], FP32, name="mv", tag="ps")[:, :F]
                emit_mv_chains(h6, mv, 0, 8)
                if copy_now:
                    emit_mv_out(mv, msg)
                return mv

            def emit_relu(ps_ap, out_ap, bias_ap):
                nonlocal relu_i
                if act_pick[relu_i % 448]:
                    nc.scalar.activation(out_ap, ps_ap, Relu, bias=bias_ap, scale=1.0)
                else:
                    nc.vector.tensor_scalar(out_ap, ps_ap, bias_ap, 0.0, ADD, MAX)
                relu_i += 1

            for sg in range(NSG):
                s0 = sg * SG
                glocal = sg // 2
                xg = [xg_t[g][sg % 2] for g in range(4)]
                if sg + 1 < NSG:
                    s1 = (sg + 1) * SG
                    for g in range(4):
                        xn = xg_t[g][(sg + 1) % 2]
                        nc.sync.dma_start(out=xn[0:8, :],
                                          in_=x3_h[g, 0:8, s1:s1 + SG])
                        nc.sync.dma_start(out=xn[64:72, :],
                                          in_=x3_h[g, 8:16, s1:s1 + SG])

                # ---- layer 0 (same 64x64 quadrant pattern as the mids) ----
                h_cur = [hpool.tile([128, SG], FP16, name=f"h{p}_a", tag=f"h{p}_a")
                         for p in range(8)]
                for g in range(4):
                    c0 = 128 * g
                    c1 = 128 * g + 64
                    for half in range(2):
                        psA = pspool.tile([128, RF], FP32, name="psA", tag="ps")
                        psB = pspool.tile([128, RF], FP32, name="psB", tag="ps")
                        # psA's 4 matmuls first (then its relu): the tile's
                        # drain can start one refill-step earlier after the
                        # ring release -- the release->refill->drain loop is
                        # what paces the whole kernel, not engine busy time.
                        for ccl in range(2):
                            cc = 2 * half + ccl
                            fs = slice(cc * F, (cc + 1) * F)
                            os_ = slice(ccl * F, (ccl + 1) * F)
                            nc.tensor.matmul(
                                out=psA[0:64, os_],
                                lhsT=w0_t[0:KX, c0:c0 + 64],
                                rhs=xg[g][0:KX, fs],
                                start=True, stop=True)
                            nc.tensor.matmul(
                                out=psA[64:128, os_],
                                lhsT=w0_t[64:64 + KX, c1:c1 + 64],
                                rhs=xg[g][64:64 + KX, fs],
                                start=True, stop=True)
                        hs = slice(half * RF, (half + 1) * RF)
                        pA, pB = 2 * g, 2 * g + 1
                        emit_relu(psA[:, :], h_cur[pA][:, hs],
                                  b0e_t[:, pA * 2 + glocal:pA * 2 + glocal + 1])
                        for ccl in range(2):
                            cc = 2 * half + ccl
                            fs = slice(cc * F, (cc + 1) * F)
                            os_ = slice(ccl * F, (ccl + 1) * F)
                            nc.tensor.matmul(
                                out=psB[64:128, os_],
                                lhsT=w0_t[0:KX, c1:c1 + 64],
                                rhs=xg[g][0:KX, fs],
                                start=True, stop=True)
                            nc.tensor.matmul(
                                out=psB[0:64, os_],
                                lhsT=w0_t[64:64 + KX, c0:c0 + 64],
                                rhs=xg[g][64:64 + KX, fs],
                                start=True, stop=True)
                        emit_relu(psB[:, :], h_cur[pB][:, hs],
                                  b0e_t[:, pB * 2 + glocal:pB * 2 + glocal + 1])

                # ---- mid layers l=0..5 ----
                for l in range(6):
                    suf = "b" if l % 2 == 0 else "a"
                    h_nxt = [hpool.tile([128, SG], FP16, name=f"h{p}_{suf}",
                                        tag=f"h{p}_{suf}") for p in range(8)]
                    for q in range(4):
                        # prior sg's matvec goes mid-l0, where its psum-tile
                        # alloc rides the ring without gating the L0->mids
                        # handoff (putting it between L0 and mids stalls both
                        # relu engines ~3us per sg boundary: the mv alloc
                        # waits an L0 drain, the chains+COPY then delay the
                        # first mids fills). Chains are split in half (~0.8us
                        # of PE each, absorbed by ring slack) and the COPY is
                        # deferred to q=3 so the drain engines never wait.
                        if l == 0 and q == 3 and mv_inflight is not None:
                            mvt, msgp, h6p = mv_inflight
                            emit_mv_chains(h6p, mvt, 4, 8)
                            emit_mv_out(mvt, msgp)
                            mv_inflight = None
                        colA = (l * 8 + 2 * q) * 64
                        colB = (l * 8 + 2 * q + 1) * 64
                        for half in range(2):
                            psA = pspool.tile([128, RF], FP32, name="psA", tag="ps")
                            psB = pspool.tile([128, RF], FP32, name="psB", tag="ps")
                            # psA first, then psB (see layer-0 comment).
                            for ccl in range(2):
                                cc = 2 * half + ccl
                                fs = slice(cc * F, (cc + 1) * F)
                                os_ = slice(ccl * F, (ccl + 1) * F)
                                nc.tensor.matmul(
                                    out=psA[0:64, os_],
                                    lhsT=wm_t[0:64, colA:colA + 64],
                                    rhs=h_cur[2 * q][0:64, fs],
                                    start=True, stop=True)
                                nc.tensor.matmul(
                                    out=psA[64:128, os_],
                                    lhsT=wm_t[64:128, colA:colA + 64],
                                    rhs=h_cur[2 * q][64:128, fs],
                                    start=True, stop=True)
                            hs = slice(half * RF, (half + 1) * RF)
                            emit_relu(psA[:, :], h_nxt[2 * q][:, hs],
                                      bm_t[:, l * 8 + 2 * q:l * 8 + 2 * q + 1])
                            for ccl in range(2):
                                cc = 2 * half + ccl
                                fs = slice(cc * F, (cc + 1) * F)
                                os_ = slice(ccl * F, (ccl + 1) * F)
                                nc.tensor.matmul(
                                    out=psB[64:128, os_],
                                    lhsT=wm_t[0:64, colB:colB + 64],
                                    rhs=h_cur[2 * q + 1][0:64, fs],
                                    start=True, stop=True)
                                nc.tensor.matmul(
                                    out=psB[0:64, os_],
                                    lhsT=wm_t[64:128, colB:colB + 64],
                                    rhs=h_cur[2 * q + 1][64:128, fs],
                                    start=True, stop=True)
                            emit_relu(psB[:, :], h_nxt[2 * q + 1][:, hs],
                                      bm_t[:, l * 8 + 2 * q + 1:l * 8 + 2 * q + 2])
                            # first half of the prior sg's matvec chains,
                            # mid-way through l0-q2 (see comment above).
                            if (l == 0 and q == 2 and half == 0
                                    and pending_mv is not None):
                                h6p, msgp = pending_mv
                                mvt = pspool.tile([128, RF], FP32, name="mv",
                                                  tag="ps")[:, :F]
                                emit_mv_chains(h6p, mvt, 0, 4)
                                mv_inflight = (mvt, msgp, h6p)
                                pending_mv = None
                    h_cur = h_nxt

                pending_mv = (h_cur, sg)
            emit_matvec(*pending_mv)
    nc.finalize()
    return nc


def kernel(xyz, joints, W0, b0, Wmid, bmid, Wf, bf):
    in_maps, c = _host_prep(xyz, joints, W0, b0, Wmid, bmid, Wf, bf)
    key = "nc"
    if key not in _CACHE:
        _CACHE[key] = _build()
    nc = _CACHE[key]
    res = run_bass_kernel_spmd(nc, in_maps, core_ids=list(range(NCORES)))
    s = np.concatenate([r["out"].reshape(-1) for r in res.results])
    return np.tanh(s + c).reshape(N, 1).astype(np.float32)



# revision 29
# speedup vs baseline: 1.1909x; 1.1909x over previous
"""Trainium2 Bass kernel for the DeformableSDFHead MLP.

Network (per point n, 16 bones k):
  x_k = [xyz3_k (3) | gl (48)]            gl shared per group of 4096 points
  h0  = relu(W0_k x_k + b0_k)             51 -> 64
  h_{l+1} = relu(Wmid_{k,l} h_l + bmid + h_l)   l = 0..6  (residual)
  latent = pre-residual out of l=6
  y = tanh(concat_k(latent_k) . Wf + bf)

Restructuring (all folds done host-side in numpy):
  * gl part of layer 0 folded into a per-(bone, group) bias beff.
  * residual folded into weights: W' = Wmid + I for l=0..5.
  * layer l=6 + final projection folded into a per-bone matvec:
      y = tanh(sum_k v_k . h6_k + c),  v_k = Wmid_{k,6}^T Wf_k.

Mapping: data-parallel over 8 cores (8192 points each). Per core, bones are
packed 2-at-a-time onto 128 partitions; mid-layer GEMMs run as 4 concurrent
64x64 quadrant tiles, layer 0 as 8-row-contraction (32,64) tiles over the
same quadrant positions. Activations fp16, psum fp32.

The kernel is PSUM-evacuation-bound: only ScalarE (1.2GHz) and VectorE
(0.96GHz) can read PSUM, at 1 elem/cycle/partition for fp32, so the
7 x 16 x 64 x 8192 relu'd activations per core set a ~255us floor on the two
engines' busy time. Everything else (PE matmuls ~210us, DMA ~30us) hides
under that. Relu+bias evacuation is split 239:209 across ScalarE (activation
Relu w/ bias) and VectorE (tensor_scalar add+max, which also carries the 4
matvec copies), matching their measured per-op rates (~1.07us vs ~1.22us per
[128,1024] tile). The per-sg matvec is scheduled mid-way through the next
sg's first mid layer -- chains split in two, PSUM-accumulator copy deferred
-- so its psum ring slot never gates the drain engines.
"""

import numpy as np

import concourse.bacc as bacc
import concourse.bass as bass
import concourse.mybir as mybir
from concourse.tile import TileContext
from concourse.bass_utils import run_bass_kernel_spmd

NUM_BONES = 16
HID = 64
JOINT_IDX = np.array([0, 1, 2, 3, 5, 6, 7, 9, 10, 11, 13, 14, 15, 17, 18, 19])

NCORES = 8
N = 65536
NS = N // NCORES       # 8192 points per core
SG = 2048              # supergroup (points held in SBUF per pipeline stage)
NSG = NS // SG         # 4
F = 512                # matmul free-dim chunk (one psum bank)
RF = 1024              # relu op free dim (2 psum banks)
KX = 8                 # layer-0 contraction: rows 0:3 bone 4g, 4:7 bone 4g+3
                       # (w0's off-slot rows are zero, so the 8-row contraction
                       # is exact; no zero-padding/memset needed -- the PE
                       # tile-mode switch to (32,64) is harmless since the
                       # matvec already switches modes every sg)

FP16 = mybir.dt.float16
FP32 = mybir.dt.float32

_SIGMA = [[(2 * p, 2 * p + 1) for p in range(8)]]
for _l in range(6):
    _SIGMA.append([_SIGMA[-1][p] if p % 2 == 0 else _SIGMA[-1][p][::-1]
                   for p in range(8)])


def _host_prep(xyz, joints, W0, b0, Wmid, bmid, Wf, bf):
    f32 = np.float32
    B = joints.shape[0]
    gl = joints[:, JOINT_IDX, :]
    gl = (gl - gl[:, :1, :]).reshape(B, -1).astype(f32)

    W0a = W0[:, :, 0:3].astype(f32)
    W0b = W0[:, :, 3:].astype(f32)
    beff = b0[:, None, :].astype(f32) + np.einsum('gi,koi->kgo', gl, W0b)

    I = np.eye(HID, dtype=f32)
    Wm_f = Wmid[:, :6].astype(f32) + I[None, None]

    Wf_k = Wf.reshape(NUM_BONES, HID).astype(f32)
    v = np.einsum('koi,ko->ki', Wmid[:, 6].astype(f32), Wf_k)
    c = float(np.sum(bmid[:, 6].astype(f32) * Wf_k) + bf[0])

    wm = np.zeros((128, 6 * 8 * 64), dtype=f32)
    bm = np.zeros((128, 48), dtype=f32)
    for l in range(6):
        for p in range(8):
            col = (l * 8 + p) * 64
            blo, bhi = _SIGMA[l][p]
            wm[0:64, col:col + 64] = Wm_f[blo, l].T
            wm[64:128, col:col + 64] = Wm_f[bhi, l].T
            olo, ohi = _SIGMA[l + 1][p]
            bm[0:64, l * 8 + p] = bmid[olo, l]
            bm[64:128, l * 8 + p] = bmid[ohi, l]

    # layer-0 weights in the same 4-quadrant layout as the mids. For group g
    # (bones 4g..4g+3), pair 2g is "straight" (diag quadrants) and pair 2g+1
    # "swapped" (anti-diag), matching _SIGMA[0]:
    #   Q(0,0)  : rhs rows 0:3   (bone 4g)   -> psA parts 0:64
    #   Q(64,64): rhs rows 64:67 (bone 4g+1) -> psA parts 64:128
    #   Q(64,0) : rhs rows 68:71 (bone 4g+2) -> psB parts 0:64
    #   Q(0,64) : rhs rows 4:7   (bone 4g+3) -> psB parts 64:128
    w0 = np.zeros((128, 512), dtype=f32)
    for g in range(4):
        w0[0:3, 128 * g:128 * g + 64] = W0a[4 * g].T
        w0[64 + 0:64 + 3, 128 * g + 64:128 * g + 128] = W0a[4 * g + 1].T
        w0[64 + 4:64 + 7, 128 * g:128 * g + 64] = W0a[4 * g + 2].T
        w0[4:7, 128 * g + 64:128 * g + 128] = W0a[4 * g + 3].T

    vt = np.zeros((128, 8 * 32), dtype=f32)
    for p in range(8):
        blo, bhi = _SIGMA[6][p]
        vt[0:64, 32 * p:32 * p + 32] = v[blo][:, None]
        vt[64:128, 32 * p:32 * p + 32] = v[bhi][:, None]

    xyzf = xyz.astype(f32)
    in_maps = []
    for core in range(NCORES):
        n0 = core * NS
        x3 = np.zeros((4, 16, NS), dtype=np.float16)
        for g in range(4):
            # rows 0:3 -> xt[0:3], 4:7 -> xt[4:7], 8:11 -> xt[64:67],
            # 12:15 -> xt[68:71]
            for slot, b_ in ((0, 4 * g), (1, 4 * g + 3),
                             (2, 4 * g + 1), (3, 4 * g + 2)):
                x3[g, 4 * slot:4 * slot + 3, :] = (
                    xyzf[n0:n0 + NS, 3 * (b_ + 1):3 * (b_ + 1) + 3].T
                    .astype(np.float16))
        b0e = np.zeros((128, 16), dtype=f32)
        for p in range(8):
            blo, bhi = _SIGMA[0][p]
            for gi in range(2):
                grp = 2 * core + gi
                b0e[0:64, p * 2 + gi] = beff[blo, grp]
                b0e[64:128, p * 2 + gi] = beff[bhi, grp]
        in_maps.append(dict(
            x3=x3,
            w0=w0.astype(np.float16),
            wm=wm.astype(np.float16),
            bm=bm,
            b0e=b0e,
            vt=vt.astype(np.float16),
        ))
    return in_maps, c


_CACHE = {}


def _build():
    nc = bacc.Bacc("TRN2", target_bir_lowering=False)

    x3_h = nc.dram_tensor("x3", [4, 16, NS], FP16, kind="ExternalInput")
    w0_h = nc.dram_tensor("w0", [128, 512], FP16, kind="ExternalInput")
    wm_h = nc.dram_tensor("wm", [128, 6 * 8 * 64], FP16, kind="ExternalInput")
    bm_h = nc.dram_tensor("bm", [128, 48], FP32, kind="ExternalInput")
    b0e_h = nc.dram_tensor("b0e", [128, 16], FP32, kind="ExternalInput")
    vt_h = nc.dram_tensor("vt", [128, 8 * 32], FP16, kind="ExternalInput")
    out_h = nc.dram_tensor("out", [NSG, 4, F], FP32, kind="ExternalOutput")

    Relu = mybir.ActivationFunctionType.Relu
    ADD = mybir.AluOpType.add
    MAX = mybir.AluOpType.max

    # relu engine split by measured rates (ACT ~1071ns vs DVE ~1215ns per op,
    # DVE also carrying the 4 matvec copies): ACT gets exactly 239 of the 448
    # relus per core, spread evenly.
    act_pick = [((i * 239) % 448) < 239 for i in range(448)]

    with TileContext(nc) as tc:
        with (
            tc.tile_pool(name="const", bufs=1) as cpool,
            tc.tile_pool(name="hbuf", bufs=2) as hpool,
            tc.tile_pool(name="outp", bufs=2) as opool,
            tc.tile_pool(name="ps", bufs=4, space="PSUM") as pspool,
        ):
            w0_t = cpool.tile([128, 512], FP16, name="w0t")
            wm_t = cpool.tile([128, 6 * 8 * 64], FP16, name="wmt")
            bm_t = cpool.tile([128, 48], FP32, name="bmt")
            b0e_t = cpool.tile([128, 16], FP32, name="b0et")
            vt_t = cpool.tile([128, 8 * 32], FP16, name="vtt")
            # persistent double-buffered x tiles: DMAs write rows 0:8 and
            # 64:72; the KX=8 contraction reads only those rows (the rest of
            # each tile is never touched).
            xg_t = [[cpool.tile([128, SG], FP16, name=f"x{g}b{b}")
                     for b in range(2)] for g in range(4)]

            nc.sync.dma_start(out=w0_t[:, :], in_=w0_h[:, :])
            nc.sync.dma_start(out=b0e_t[:, :], in_=b0e_h[:, :])
            for g in range(4):
                nc.sync.dma_start(out=xg_t[g][0][0:8, :],
                                  in_=x3_h[g, 0:8, 0:SG])
                nc.sync.dma_start(out=xg_t[g][0][64:72, :],
                                  in_=x3_h[g, 8:16, 0:SG])
            for l in range(6):
                nc.sync.dma_start(out=wm_t[:, 512 * l:512 * (l + 1)],
                                  in_=wm_h[:, 512 * l:512 * (l + 1)])
            nc.sync.dma_start(out=bm_t[:, :], in_=bm_h[:, :])
            nc.sync.dma_start(out=vt_t[:, :], in_=vt_h[:, :])

            relu_i = 0
            mv_inflight = None
            pending_mv = None  # delay matvec past next sg's layer 0 so the
                               # PE stays fed while the prior sg's last relus
                               # drain; the L0 matmuls (which depend on
                               # nothing) must all sit BEFORE the matvec in
                               # the in-order PE queue or they get blocked
                               # behind its relu-wait and starve ACT/DVE

            def emit_mv_chains(h6, mv, p_lo, p_hi):
                # 4 col-concurrent accumulation chains (one per cc), p-outer
                # so the chains interleave and overlap in the PE array.
                for p in range(p_lo, p_hi):
                    for cc in range(4):
                        nc.tensor.matmul(
                            out=mv[32 * cc:32 * cc + 32, :],
                            lhsT=vt_t[:, 32 * p:32 * p + 32],
                            rhs=h6[p][:, cc * F:(cc + 1) * F],
                            start=(p == 0), stop=(p == 7),
                            tile_position=(0, 32 * cc),
                            skip_group_check=True)

            def emit_mv_out(mv, msg):
                # on DVE: cheaper there than ACT's activation-copy (~617 vs
                # ~686ns) and ACT is the busier engine.
                out_sb = opool.tile([128, F], FP32, name="osb", tag="osb")
                nc.vector.tensor_copy(out_sb[0:97, :], mv[0:97, :])
                ou_v = out_sb.rearrange("(a b) f -> a b f", b=32)[:, 0:1, :]
                nc.sync.dma_start(out=out_h[msg, :, :], in_=ou_v)

            def emit_matvec(h6, msg, mv=None, copy_now=True):
                if mv is None:
                    mv = pspool.tile([128, RF], FP32, name="mv", tag="ps")[:, :F]
                emit_mv_chains(h6, mv, 0, 8)
                if copy_now:
                    emit_mv_out(mv, msg)
                return mv

            def emit_relu(ps_ap, out_ap, bias_ap):
                nonlocal relu_i
                if act_pick[relu_i % 448]:
                    nc.scalar.activation(out_ap, ps_ap, Relu, bias=bias_ap, scale=1.0)
                else:
                    nc.vector.tensor_scalar(out_ap, ps_ap, bias_ap, 0.0, ADD, MAX)
                relu_i += 1

            for sg in range(NSG):
                s0 = sg * SG
                glocal = sg // 2
                xg = [xg_t[g][sg % 2] for g in range(4)]
                if sg + 1 < NSG:
                    s1 = (sg + 1) * SG
                    for g in range(4):
                        xn = xg_t[g][(sg + 1) % 2]
                        nc.sync.dma_start(out=xn[0:8, :],
                                          in_=x3_h[g, 0:8, s1:s1 + SG])
                        nc.sync.dma_start(out=xn[64:72, :],
                                          in_=x3_h[g, 8:16, s1:s1 + SG])

                # ---- layer 0 (same 64x64 quadrant pattern as the mids) ----
                h_cur = [hpool.tile([128, SG], FP16, name=f"h{p}_a", tag=f"h{p}_a")
                         for p in range(8)]
                for g in range(4):
                    c0 = 128 * g
                    c1 = 128 * g + 64
                    for half in range(2):
                        psA = pspool.tile([128, RF], FP32, name="psA", tag="ps")
                        psB = pspool.tile([128, RF], FP32, name="psB", tag="ps")
                        # psA's 4 matmuls first (then its relu): the tile's
                        # drain can start one refill-step earlier after the
                        # ring release -- the release->refill->drain loop is
                        # what paces the whole kernel, not engine busy time.
                        for ccl in range(2):
                            cc = 2 * half + ccl
                            fs = slice(cc * F, (cc + 1) * F)
                            os_ = slice(ccl * F, (ccl + 1) * F)
                            nc.tensor.matmul(
                                out=psA[0:64, os_],
                                lhsT=w0_t[0:KX, c0:c0 + 64],
                                rhs=xg[g][0:KX, fs],
                                start=True, stop=True)
                            nc.tensor.matmul(
                                out=psA[64:128, os_],
                                lhsT=w0_t[64:64 + KX, c1:c1 + 64],
                                rhs=xg[g][64:64 + KX, fs],
                                start=True, stop=True)
                        hs = slice(half * RF, (half + 1) * RF)
                        pA, pB = 2 * g, 2 * g + 1
                        emit_relu(psA[:, :], h_cur[pA][:, hs],
                                  b0e_t[:, pA * 2 + glocal:pA * 2 + glocal + 1])
                        for ccl in range(2):
                            cc = 2 * half + ccl
                            fs = slice(cc * F, (cc + 1) * F)
                            os_ = slice(ccl * F, (ccl + 1) * F)
                            nc.tensor.matmul(
                                out=psB[64:128, os_],
                                lhsT=w0_t[0:KX, c1:c1 + 64],
                                rhs=xg[g][0:KX, fs],
                                start=True, stop=True)
                            nc.tensor.matmul(
                                out=psB[0:64, os_],
                                lhsT=w0_t[64:64 + KX, c0:c0 + 64],
                                rhs=xg[g][64:64 + KX, fs],
                                start=True, stop=True)
                        emit_relu(psB[:, :], h_cur[pB][:, hs],
                                  b0e_t[:, pB * 2 + glocal:pB * 2 + glocal + 1])

                # ---- mid layers l=0..5 ----
                for l in range(6):
                    suf = "b" if l % 2 == 0 else "a"
                    h_nxt = [hpool.tile([128, SG], FP16, name=f"h{p}_{suf}",
                                        tag=f"h{p}_{suf}") for p in range(8)]
                    for q in range(4):
                        # prior sg's matvec goes mid-l0, where its psum-tile
                        # alloc rides the ring without gating the L0->mids
                        # handoff (putting it between L0 and mids stalls both
                        # relu engines ~3us per sg boundary: the mv alloc
                        # waits an L0 drain, the chains+COPY then delay the
                        # first mids fills). Chains are split in half (~0.8us
                        # of PE each, absorbed by ring slack) and the COPY is
                        # deferred to q=3 so the drain engines never wait.
                        if l == 0 and q == 3 and mv_inflight is not None:
                            mvt, msgp, h6p = mv_inflight
                            emit_mv_chains(h6p, mvt, 4, 8)
                            emit_mv_out(mvt, msgp)
                            mv_inflight = None
                        colA = (l * 8 + 2 * q) * 64
                        colB = (l * 8 + 2 * q + 1) * 64
                        for half in range(2):
                            psA = pspool.tile([128, RF], FP32, name="psA", tag="ps")
                            psB = pspool.tile([128, RF], FP32, name="psB", tag="ps")
                            # psA first, then psB (see layer-0 comment).
                            for ccl in range(2):
                                cc = 2 * half + ccl
                                fs = slice(cc * F, (cc + 1) * F)
                                os_ = slice(ccl * F, (ccl + 1) * F)
                                nc.tensor.matmul(
                                    out=psA[0:64, os_],
                                    lhsT=wm_t[0:64, colA:colA + 64],
                                    rhs=h_cur[2 * q][0:64, fs],
                                    start=True, stop=True)
                                nc.tensor.matmul(
                                    out=psA[64:128, os_],
                                    lhsT=wm_t[64:128, colA:colA + 64],
                                    rhs=h_cur[2 * q][64:128, fs],
                                    start=True, stop=True)
                            hs = slice(half * RF, (half + 1) * RF)
                            emit_relu(psA[:, :], h_nxt[2 * q][:, hs],
                                      bm_t[:, l * 8 + 2 * q:l * 8 + 2 * q + 1])
                            for ccl in range(2):
                                cc = 2 * half + ccl
                                fs = slice(cc * F, (cc + 1) * F)
                                os_ = slice(ccl * F, (ccl + 1) * F)
                                nc.tensor.matmul(
                                    out=psB[64:128, os_],
                                    lhsT=wm_t[0:64, colB:colB + 64],
                                    rhs=h_cur[2 * q + 1][0:64, fs],
                                    start=True, stop=True)
                                nc.tensor.matmul(
                                    out=psB[0:64, os_],
                                    lhsT=wm_t[64:128, colB:colB + 64],
                                    rhs=h_cur[2 * q + 1][64:128, fs],
                                    start=True, stop=True)
                            emit_relu(psB[:, :], h_nxt[2 * q + 1][:, hs],
                                      bm_t[:, l * 8 + 2 * q + 1:l * 8 + 2 * q + 2])
                            # first half of the prior sg's matvec chains,
                            # mid-way through l0-q2 (see comment above).
                            if (l == 0 and q == 2 and half == 0
                                    and pending_mv is not None):
                                h6p, msgp = pending_mv
                                mvt = pspool.tile([128, RF], FP32, name="mv",
                                                  tag="ps")[:, :F]
                                emit_mv_chains(h6p, mvt, 0, 4)
                                mv_inflight = (mvt, msgp, h6p)
                                pending_mv = None
                    h_cur = h_nxt

                pending_mv = (h_cur, sg)
            emit_matvec(*pending_mv)
    nc.finalize()
    return nc


def kernel(xyz, joints, W0, b0, Wmid, bmid, Wf, bf):
    in_maps, c = _host_prep(xyz, joints, W0, b0, Wmid, bmid, Wf, bf)
    key = "nc"
    if key not in _CACHE:
        _CACHE[key] = _build()
    nc = _CACHE[key]
    res = run_bass_kernel_spmd(nc, in_maps, core_ids=list(range(NCORES)))
    s = np.concatenate([r["out"].reshape(-1) for r in res.results])
    return np.tanh(s + c).reshape(N, 1).astype(np.float32)

